# revision 44
# baseline (speedup 1.0000x reference)
"""ChebConv(K=2) x2 + BatchNorm + LeakyReLU + global_mean_pool + linear head
on 8 Trainium2 NeuronCores.

Sharding: edges partitioned by destination node (col) across cores; node
features replicated in DRAM (bf16) for per-edge gathering; per-graph pooling
via one-hot matmul built on-device, partials combined on host.

bf16 data path: gather tables, scatter one-hot/weights, and all matmuls run
in bf16 (PSUM accumulation fp32); BatchNorm stats and the final head stay
fp32.

Self-contained: only needs the container's `concourse` package.
"""
import math
import numpy as np
from contextlib import ExitStack

import ml_dtypes

import concourse.bass as bass
import concourse.tile as tile
from concourse import mybir, library_config
from concourse.bass_utils import run_bass_kernel_spmd
from concourse.masks import make_identity

P = 128          # partitions / edges per chunk
BQ = 8           # max chunks per gather call (pow2 blocks)
SUB = 32         # dst subtile width (sel width)
SPW = 15         # subtiles per PSUM window (480 dst nodes + 32 slack = 512)
IDX_SPLIT = 32768  # int16 gather index limit -> two source groups
F = 64           # feature width (both layers)
NCORES = 8
NOGATHER = False  # differential profiling switch
FREERUN = False   # diagnostic: issue gathers but feed matmuls from a dummy
PREFETCH = 0      # extra windows of gather blocks issued ahead
L2_FROM_X = False  # diagnostic: layer-2 gathers read the x table, not h_full
COPY_SEGS = 8      # h_full -> h_loc staging copy segments (round-robin engines)
AG_CHUNKS = 1      # AllGather split count (pipelines collective with staging)
NQUEUES = 4      # SWDGE queues (max 4)
VAL_BUFS = 24    # gather tile pool depth
DMA_SCRATCH = 16384  # SWDGE descriptor ring bytes (//16 = ring descs)
SINGLE_PACKET = True

FP = mybir.dt.float32
BF = mybir.dt.bfloat16
I16 = mybir.dt.int16
NPBF = ml_dtypes.bfloat16
TW = 128         # padded gather-table row width (stride 256B in bf16)


def _dma_gather_raw(eng, out_ap, in_ap, idxs_ap, num_idxs, num_idxs_reg,
                    elem_size, elem_step, queue_num=0):
    """dma_gather with elem_size_bytes=128 (sub-256B payload) on a 256B-stride
    table. Mirrors BassGpSimd.dma_gather minus its elem_size%256 assert (the
    stride, which the ISA encodes in 256B units, stays a 256B multiple)."""
    from concourse import ap_utils
    from concourse.bass import MemorySpace

    assert idxs_ap.dtype == mybir.dt.int16
    assert in_ap.dtype == out_ap.dtype
    assert in_ap.space == MemorySpace.DRAM
    assert idxs_ap.space == MemorySpace.SBUF
    assert out_ap.space == MemorySpace.SBUF
    assert ap_utils.ap_is_contiguous(out_ap.ap[1:])
    assert ap_utils.ap_is_contiguous(idxs_ap.ap[1:])
    assert in_ap.ap[-1][1] == out_ap.ap[-1][1] == elem_size
    assert in_ap.ap[0][0] == elem_step
    stride_bytes = elem_step * mybir.dt.size(in_ap.dtype)
    assert stride_bytes % 256 == 0
    stride_bytes_256 = stride_bytes // 256
    _in_ap = eng.lower_ap_dma(in_ap, for_custom_bir_dma=True)
    inst = eng.add_instruction(
        mybir.InstDMAGatherAnt(
            name=eng.bass.get_next_instruction_name(),
            ins=[*_in_ap, eng.lower_ap(idxs_ap),
                 eng.lower_val_access(eng.to_reg(num_idxs_reg))],
            outs=[eng.lower_ap(out_ap)],
            transpose=False,
            num_idxs=num_idxs,
            elem_size=elem_size,
            stride_bytes_256=stride_bytes_256,
            gen_mode=0,
            single_packet=SINGLE_PACKET,
            queue_num=queue_num,
            sbuf_tokens_per_rank=0,
            sbuf_free_dim_per_rank=0,
            sbuf_free_dim_pad_per_rank=0,
            sbuf_byte_offset=0,
        )
    )
    return inst


# ---------------------------------------------------------------------------
# BIR post-passes (this container's walrus accepts only one sync wait per
# instruction, and never lowers InstPseudoReloadLibraryIndex itself).
# ---------------------------------------------------------------------------
_CTR = [0]


def _fix_reload_order(nc):
    """Tile schedules dep-free instructions eagerly; move the final
    standard-lib reload after the last DMAGatherAnt."""
    for f in nc.m.functions:
        for bb in f.blocks:
            insts = list(bb.instructions)
            std_i = [i for i, it in enumerate(insts)
                     if getattr(it, "op_name", None) == "PseudoReloadLibraryIndex"
                     and it.lib_index == 0]
            gat_i = [i for i, it in enumerate(insts)
                     if type(it).__name__ == "InstDMAGatherAnt"]
            if std_i and gat_i and std_i[0] < gat_i[-1]:
                reload_inst = insts.pop(std_i[0])
                insts.insert(gat_i[-1], reload_inst)
                bb.instructions = insts


def _finalize_bir(nc):
    for f in nc.m.functions:
        for bb in f.blocks:
            out = []
            changed = False
            for inst in bb.instructions:
                if getattr(inst, "op_name", None) == "PseudoReloadLibraryIndex":
                    instr = [0] * 64
                    instr[0] = 223
                    instr[1] = 16
                    instr[12] = 2
                    instr[16] = inst.lib_index
                    inst.instr = instr
                si = inst.sync_info
                if si is not None and si.on_wait is not None and len(si.on_wait) > 1:
                    changed = True
                    for w in si.on_wait[:-1]:
                        _CTR[0] += 1
                        nop = mybir.InstNoOp(
                            name=f"waitnop-{_CTR[0]}",
                            engine=inst.engine,
                            sync_info=mybir.SyncInfo(on_wait=[w], on_update=[]),
                        )
                        out.append(nop)
                    inst.sync_info = mybir.SyncInfo(
                        on_wait=[si.on_wait[-1]], on_update=si.on_update
                    )
                out.append(inst)
            if changed:
                bb.instructions = out


# ---------------------------------------------------------------------------
# Host-side planning: bucket edges by (core, dst subtile, src group), build
# the static chunk layout (max chunk count across cores per bucket) and the
# per-core packed idx/w/rel arrays.
# ---------------------------------------------------------------------------
class Plan:
    pass


def _plan(edge_index, batch, x, G):
    N = x.shape[0]
    E = edge_index.shape[1]
    NLOC = (N + NCORES - 1) // NCORES
    assert N == NLOC * NCORES, "node count must split evenly"
    NSUB = (NLOC + SUB - 1) // SUB
    NWIN = (NSUB + SPW - 1) // SPW

    row = np.asarray(edge_index[0], dtype=np.int64)
    col = np.asarray(edge_index[1], dtype=np.int64)
    deg = np.bincount(row, minlength=N).astype(np.float64)
    dis = np.where(deg > 0, deg ** -0.5, 0.0)
    w_all = (-(dis[row] * dis[col])).astype(np.float32)

    core_of = col // NLOC
    local = col - core_of * NLOC
    sub = local // SUB
    grp = (row >= IDX_SPLIT).astype(np.int64)

    key = (core_of * NSUB + sub) * 2 + grp
    counts = np.bincount(key, minlength=NCORES * NSUB * 2).reshape(NCORES, NSUB, 2)
    K = np.ceil(counts.max(axis=0) / P).astype(np.int64)  # [NSUB, 2]

    # chunk column layout ordered (group, window, subtile): each group's
    # columns are contiguous so gather blocks pack to the full BQ*P
    # descriptor budget regardless of window boundaries
    col_index = np.zeros((NSUB, 2), np.int64)
    spans = []  # (wi, g, col_start, ncols)
    group_base = {}
    T = 0
    for g in (0, 1):
        group_base[g] = T
        for wi in range(NWIN):
            subs = range(wi * SPW, min((wi + 1) * SPW, NSUB))
            start = T
            for s in subs:
                col_index[s, g] = T
                T += K[s, g]
            spans.append((wi, g, start, T - start))
    group_size = {0: group_base[1], 1: T - group_base[1]}

    # per-edge placement
    order = np.argsort(key, kind="stable")
    kk = key[order]
    bucket_first = np.r_[0, np.flatnonzero(np.diff(kk)) + 1]
    sizes = np.diff(np.r_[bucket_first, E])
    j = np.arange(E) - np.repeat(bucket_first, sizes)  # rank within bucket

    m_o = core_of[order]
    c_o = col_index[sub[order], grp[order]] + j // P
    p_o = j % P
    idx_loc = np.where(grp == 0, row, row - IDX_SPLIT).astype(np.int16)[order]
    w_o = w_all[order]
    rel_o = (local - sub * SUB).astype(np.float32)[order]

    w_pc = np.zeros((NCORES, P, T), np.float32)
    rel_pc = np.zeros((NCORES, P, T), np.float32)
    idx_pc = np.zeros((NCORES, 16, 8 * T), np.int16)
    w_pc[m_o, p_o, c_o] = w_o
    rel_pc[m_o, p_o, c_o] = rel_o
    idx_pc[m_o, p_o % 16, 8 * c_o + p_o // 16] = idx_loc
    idx_full = np.tile(idx_pc, (1, 8, 1))  # [NCORES, 128, 8T]

    pl = Plan()
    pl.N, pl.E, pl.G = N, E, G
    pl.NLOC, pl.NSUB, pl.NWIN, pl.T = NLOC, NSUB, NWIN, T
    pl.K, pl.col_index, pl.spans = K, col_index, spans
    pl.group_base, pl.group_size = group_base, group_size
    pl.w_pc, pl.rel_pc, pl.idx_full = w_pc, rel_pc, idx_full
    pl.empty_subs = [s for s in range(NSUB) if K[s, 0] + K[s, 1] == 0]
    pl.batch = np.asarray(batch, dtype=np.int64)
    pl.cnts = np.bincount(pl.batch, minlength=G).astype(np.float32)
    return pl


# ---------------------------------------------------------------------------
# Device program
# ---------------------------------------------------------------------------
def _emit_cheb(nc, tc, ctx, pl, pools, tiles, table_ap, wstack_t, bias_t, h_out,
               h_dtype):
    """One Cheb layer: scatter (gather + sel matmuls into PSUM windows) into
    stacked[64:128], then dense matmul with [x_T; Tx_T] and bias add into
    h_out[64, NLOC]."""
    NLOC, NSUB, NWIN, K = pl.NLOC, pl.NSUB, pl.NWIN, pl.K
    valp, selp, psw, psd = pools["val"], pools["sel"], pools["psw"], pools["psd"]
    stacked, idx_t, w_t, rel_t, iota_f = (
        tiles["stacked"], tiles["idx"], tiles["w"], tiles["rel"], tiles["iota_f"],
    )

    spans_by_win = {}
    for (wi, g, start, ncols) in pl.spans:
        spans_by_win.setdefault(wi, []).append((g, start, ncols))
    gbase, gsize = pl.group_base, pl.group_size

    # gather blocks stream over each group's contiguous column range in
    # full-BQ chunks, decoupled from window boundaries; issued lazily when
    # the first window touching them is processed
    blocks = {}  # (g, bid) -> (abs base col, tile)

    def ensure_block(g, bid):
        if (g, bid) in blocks:
            return
        b0 = gbase[g] + bid * BQ
        bw = min(BQ, gbase[g] + gsize[g] - b0)
        bt = valp.tile([P, BQ, F], BF, tag="val")
        if NOGATHER:
            nc.vector.memset(bt[:, :bw, :], 0.5)
        else:
            src = table_ap[:, :F] if g == 0 else table_ap[IDX_SPLIT:, :F]
            _dma_gather_raw(
                nc.gpsimd, out_ap=bt[:, :bw, :], in_ap=src,
                idxs_ap=idx_t[:, 8 * b0: 8 * (b0 + bw)],
                num_idxs=bw * P, num_idxs_reg=tiles["ni_regs"][bw * P],
                elem_size=F, elem_step=TW,
                queue_num=tiles["qrr"][0] % NQUEUES,
            )
        tiles["qrr"][0] += 1
        blocks[(g, bid)] = (b0, bt)

    dummy = None
    if FREERUN:
        dummy = valp.tile([P, BQ, F], BF, tag="dummy")
        nc.vector.memset(dummy[:], 0.5)

    for wi in range(NWIN):
        ps_w = psw.tile([F, 512], FP, tag="psw")
        win_subs = range(wi * SPW, min((wi + 1) * SPW, NSUB))
        sel_tiles = {}
        for wj in range(wi, min(wi + 1 + PREFETCH, NWIN)):
            for (g, start, ncols) in spans_by_win[wj]:
                if ncols == 0:
                    continue
                r0 = start - gbase[g]
                for bid in range(r0 // BQ, -(-(r0 + ncols) // BQ)):
                    ensure_block(g, bid)
        for (g, start, ncols) in spans_by_win[wi]:
            if ncols == 0:
                continue
            sel_t = selp.tile([P, ncols, SUB], BF, tag="sel")
            nc.vector.tensor_tensor(
                out=sel_t[:],
                in0=rel_t[:, start:start + ncols, None].broadcast_to([P, ncols, SUB]),
                in1=iota_f[:, None, :].broadcast_to([P, ncols, SUB]),
                op=mybir.AluOpType.is_equal,
            )
            nc.vector.tensor_tensor(
                out=sel_t[:], in0=sel_t[:],
                in1=w_t[:, start:start + ncols, None].broadcast_to([P, ncols, SUB]),
                op=mybir.AluOpType.mult,
            )
            sel_tiles[g] = (start, sel_t)

        # chunk matmuls, accumulate into the window PSUM; the first matmul
        # per subtile carries start=True (zero-initializes its PSUM slice)
        mms = []
        for s in win_subs:
            boff = SUB * (s - wi * SPW)
            for g in (0, 1):
                if K[s, g] == 0 or g not in sel_tiles:
                    continue
                for r in range(K[s, g]):
                    mms.append((s, boff, g, pl.col_index[s, g] + r))
        started = set()
        for i, (s, boff, g, cc) in enumerate(mms):
            b0, bt = blocks[(g, (cc - gbase[g]) // BQ)]
            if FREERUN:
                lhs = dummy[:, (cc - b0) % BQ, :]
            else:
                lhs = bt[:, cc - b0, :]
            span_start, sel_t = sel_tiles[g]
            nc.tensor.matmul(
                out=ps_w[:, boff:boff + SUB],
                lhsT=lhs, rhs=sel_t[:, cc - span_start, :],
                start=(s not in started), stop=(i == len(mms) - 1),
            )
            started.add(s)
        # copy the window's real columns into stacked[64:128]
        lo = wi * SPW * SUB
        hi = min(lo + SPW * SUB, NLOC)
        nc.scalar.copy(out=stacked[F:2 * F, lo:hi], in_=ps_w[:, :hi - lo])
        for s in pl.empty_subs:
            if wi * SPW <= s < min((wi + 1) * SPW, NSUB):
                slo, shi = s * SUB, min((s + 1) * SUB, NLOC)
                nc.vector.memset(stacked[F:2 * F, slo:shi], 0.0)

    # dense: h = Wstack.T @ [x_T; Tx_T] + b
    nspan = 512
    for i in range(0, NLOC, nspan):
        wdt = min(nspan, NLOC - i)
        ps_d = psd.tile([F, 512], FP, tag="psd")
        nc.tensor.matmul(out=ps_d[:, :wdt], lhsT=wstack_t[:],
                         rhs=stacked[:, i:i + wdt], start=True, stop=True)
        nc.scalar.activation(h_out[:, i:i + wdt], ps_d[:, :wdt],
                             mybir.ActivationFunctionType.Identity, bias=bias_t[:])


def _build_program(pl, num_devices=NCORES, no_cc=False, repeat=1):
    N, NLOC, G, T = pl.N, pl.NLOC, pl.G, pl.T
    NT = (NLOC + P - 1) // P
    nc = bass.Bass("TRN2", target_bir_lowering=False, debug=False,
                   num_devices=num_devices, num_swdge_queues=NQUEUES,
                   dynamic_dma_scratch_size=DMA_SCRATCH)

    xbf_in = nc.dram_tensor("xbf_in", [N, TW], BF, kind="ExternalInput").ap()
    xT_in = nc.dram_tensor("xT_in", [F, NLOC], BF, kind="ExternalInput").ap()
    idx_in = nc.dram_tensor("idx_in", [P, 8 * T], I16, kind="ExternalInput").ap()
    w_in = nc.dram_tensor("w_in", [P, T], BF, kind="ExternalInput").ap()
    rel_in = nc.dram_tensor("rel_in", [P, T], BF, kind="ExternalInput").ap()
    w1_in = nc.dram_tensor("w1_in", [2 * F, F], BF, kind="ExternalInput").ap()
    w2_in = nc.dram_tensor("w2_in", [2 * F, F], BF, kind="ExternalInput").ap()
    b1_in = nc.dram_tensor("b1_in", [F, 1], FP, kind="ExternalInput").ap()
    b2_in = nc.dram_tensor("b2_in", [F, 1], FP, kind="ExternalInput").ap()
    gam_in = nc.dram_tensor("gam_in", [F, 1], FP, kind="ExternalInput").ap()
    bet_in = nc.dram_tensor("bet_in", [F, 1], FP, kind="ExternalInput").ap()
    lw_in = nc.dram_tensor("lw_in", [P, F], FP, kind="ExternalInput").ap()
    bat_in = nc.dram_tensor("bat_in", [P, NT], BF, kind="ExternalInput").ap()
    out_d = nc.dram_tensor("out_d", [G, 1], FP, kind="ExternalOutput").ap()

    h_slab = nc.dram_tensor("h_slab", [NLOC, TW], BF).ap()
    h_full = nc.dram_tensor("h_full", [N, TW], BF, addr_space="Shared").ap()
    h_loc = nc.dram_tensor("h_loc", [N, TW], BF).ap()
    st_in = nc.dram_tensor("st_in", [F, 2], FP).ap()
    st_out = nc.dram_tensor("st_out", [F, 2], FP, addr_space="Shared").ap()

    with tile.TileContext(nc) as tc, ExitStack() as ctx:
        cst = ctx.enter_context(tc.tile_pool(name="cst", bufs=1))
        big = ctx.enter_context(tc.tile_pool(name="big", bufs=1))
        hbuf = ctx.enter_context(tc.tile_pool(name="hbuf", bufs=1))
        valp = ctx.enter_context(tc.tile_pool(name="valp", bufs=VAL_BUFS))
        selp = ctx.enter_context(tc.tile_pool(name="selp", bufs=4))
        mp = ctx.enter_context(tc.tile_pool(name="mp", bufs=2))
        sml = ctx.enter_context(tc.tile_pool(name="sml", bufs=1))
        psw = ctx.enter_context(tc.tile_pool(name="psw", bufs=4, space="PSUM"))
        psd = ctx.enter_context(tc.tile_pool(name="psd", bufs=1, space="PSUM"))
        pst = ctx.enter_context(tc.tile_pool(name="pst", bufs=2, space="PSUM"))
        psp = ctx.enter_context(tc.tile_pool(name="psp", bufs=1, space="PSUM"))
        pools = {"val": valp, "sel": selp, "psw": psw, "psd": psd}

        # --- constants & inputs (standard ucode ops before mlp lib load) ---
        iota_i = cst.tile([P, SUB], mybir.dt.int32)
        nc.gpsimd.iota(iota_i[:], pattern=[[1, SUB]], base=0, channel_multiplier=0)
        iota_f = cst.tile([P, SUB], BF)
        nc.vector.tensor_copy(out=iota_f[:], in_=iota_i[:])
        iota_gi = cst.tile([P, G], mybir.dt.int32)
        nc.gpsimd.iota(iota_gi[:], pattern=[[1, G]], base=0, channel_multiplier=0)
        iota_g = cst.tile([P, G], BF)
        nc.vector.tensor_copy(out=iota_g[:], in_=iota_gi[:])
        ident = cst.tile([F, F], BF)
        make_identity(nc, ident[:])
        nc.gpsimd.load_library(library_config.mlp)

        idx_t = cst.tile([P, 8 * T], I16)
        nc.sync.dma_start(out=idx_t[:], in_=idx_in[:])
        w_t = cst.tile([P, T], BF)
        nc.sync.dma_start(out=w_t[:], in_=w_in[:])
        rel_t = cst.tile([P, T], BF)
        nc.sync.dma_start(out=rel_t[:], in_=rel_in[:])
        w1_t = cst.tile([2 * F, F], BF)
        nc.sync.dma_start(out=w1_t[:], in_=w1_in[:])
        w2_t = cst.tile([2 * F, F], BF)
        nc.sync.dma_start(out=w2_t[:], in_=w2_in[:])
        b1_t = cst.tile([F, 1], FP)
        nc.sync.dma_start(out=b1_t[:], in_=b1_in[:])
        b2_t = cst.tile([F, 1], FP)
        nc.sync.dma_start(out=b2_t[:], in_=b2_in[:])
        gam_t = cst.tile([F, 1], FP)
        nc.sync.dma_start(out=gam_t[:], in_=gam_in[:])
        bet_t = cst.tile([F, 1], FP)
        nc.sync.dma_start(out=bet_t[:], in_=bet_in[:])
        lw_t = cst.tile([P, F], FP)
        nc.sync.dma_start(out=lw_t[:], in_=lw_in[:])
        bat_t = cst.tile([P, NT], BF)
        nc.sync.dma_start(out=bat_t[:], in_=bat_in[:])

        stacked = big.tile([P, NLOC], BF)
        ni_regs = {}
        for r in range(1, BQ + 1):
            v = r * P
            reg = nc.gpsimd.alloc_register(f"ni{v}")
            nc.gpsimd.reg_mov(reg, v)
            ni_regs[v] = reg
        tiles = {"stacked": stacked, "idx": idx_t, "w": w_t, "rel": rel_t,
                 "iota_f": iota_f, "ni_regs": ni_regs, "qrr": [0]}

        for _rep in range(repeat):
            # --- layer 1 ---
            nc.sync.dma_start(out=stacked[:F, :], in_=xT_in[:])
            h_pre = hbuf.tile([F, NLOC], FP, tag="hpre")
            _emit_cheb(nc, tc, ctx, pl, pools, tiles, xbf_in, w1_t, b1_t,
                       h_pre[:], FP)

            # --- BN stats + AllReduce ---
            sum_t = sml.tile([F, 1], FP, tag="sum")
            nc.vector.tensor_reduce(out=sum_t[:], in_=h_pre[:],
                                    axis=mybir.AxisListType.X,
                                    op=mybir.AluOpType.add)
            scratch = hbuf.tile([F, NLOC], FP, tag="scratch2")
            sumsq_t = sml.tile([F, 1], FP, tag="sumsq")
            nc.scalar.activation(scratch[:], h_pre[:],
                                 mybir.ActivationFunctionType.Square,
                                 accum_out=sumsq_t[:])
            st_t = sml.tile([F, 2], FP, tag="st")
            nc.vector.tensor_copy(out=st_t[:, 0:1], in_=sum_t[:])
            nc.vector.tensor_copy(out=st_t[:, 1:2], in_=sumsq_t[:])
            nc.sync.dma_start(out=st_in[:], in_=st_t[:])
            if no_cc:
                nc.sync.dma_start(out=st_out[:], in_=st_in[:])
            else:
                nc.gpsimd.collective_compute(
                    "AllReduce", mybir.AluOpType.add,
                    replica_groups=[list(range(num_devices))],
                    ins=[st_in[:]], outs=[st_out[:]],
                )
            str_t = sml.tile([F, 2], FP, tag="str")
            nc.sync.dma_start(out=str_t[:], in_=st_out[:])

            # s = gamma * rsqrt(var + eps); t = beta - mu * s
            invN = 1.0 / float(N)
            mu_t = sml.tile([F, 1], FP, tag="mu")
            nc.vector.tensor_scalar_mul(mu_t[:], str_t[:, 0:1], invN)
            msq_t = sml.tile([F, 1], FP, tag="msq")
            nc.vector.tensor_scalar_mul(msq_t[:], str_t[:, 1:2], invN)
            var_t = sml.tile([F, 1], FP, tag="var")
            nc.vector.scalar_tensor_tensor(out=var_t[:], in0=mu_t[:],
                                           scalar=-1.0, in1=mu_t[:],
                                           op0=mult_op(), op1=mult_op())
            nc.vector.tensor_add(var_t[:], var_t[:], msq_t[:])
            eps_t = sml.tile([F, 1], FP, tag="eps")
            nc.vector.memset(eps_t[:], 1e-5)
            sd_t = sml.tile([F, 1], FP, tag="sd")
            nc.scalar.activation(sd_t[:], var_t[:],
                                 mybir.ActivationFunctionType.Sqrt,
                                 bias=eps_t[:])
            rs_t = sml.tile([F, 1], FP, tag="rs")
            nc.vector.reciprocal(rs_t[:], sd_t[:])
            s_t = sml.tile([F, 1], FP, tag="s")
            nc.vector.tensor_mul(s_t[:], gam_t[:], rs_t[:])
            t_t = sml.tile([F, 1], FP, tag="t")
            nc.vector.tensor_mul(t_t[:], mu_t[:], s_t[:])
            nc.vector.tensor_sub(t_t[:], bet_t[:], t_t[:])

            # h = lrelu(h_pre * s + t) -> stacked[:64] (bf16)
            z_t = hbuf.tile([F, NLOC], FP, tag="scratch2")
            nc.vector.tensor_scalar(out=z_t[:], in0=h_pre[:], scalar1=s_t[:],
                                    scalar2=t_t[:], op0=mult_op(),
                                    op1=add_op())
            nc.vector.scalar_tensor_tensor(out=stacked[:F, :], in0=z_t[:],
                                           scalar=0.01, in1=z_t[:],
                                           op0=mult_op(), op1=max_op())

            # transpose h -> h_slab (node-major bf16) and AllGather; the
            # bf16 transpose writes PSUM in bf16, so DMA straight from PSUM
            for i in range(0, NLOC, P):
                wdt = min(P, NLOC - i)
                ps_b = pst.tile([P, F], BF, tag="pstb")
                nc.tensor.matmul(out=ps_b[:wdt, :], lhsT=stacked[:F, i:i + wdt],
                                 rhs=ident[:], start=True, stop=True,
                                 is_transpose=True)
                hnm_t = mp.tile([P, F], BF, tag="hnm")
                nc.vector.tensor_copy(out=hnm_t[:wdt, :], in_=ps_b[:wdt, :])
                nc.sync.dma_start(out=h_slab[i:i + wdt, :F], in_=hnm_t[:wdt, :])
            # AllGather h, then stage the table into local DRAM (Shared-space
            # random reads are ~5x slower than local DRAM). AG_CHUNKS > 1
            # pipelines the collective chunks with the staging copies.
            agc = AG_CHUNKS
            ch = NLOC // agc
            assert NLOC % agc == 0
            engs = [nc.sync, nc.scalar, nc.gpsimd]
            ei = 0
            for k in range(agc):
                slab_k = h_slab[k * ch:(k + 1) * ch, :]
                full_k = h_full[k * ch * NCORES:(k + 1) * ch * NCORES, :]
                if no_cc:
                    for _r in range(NCORES):
                        nc.sync.dma_start(
                            out=full_k[_r * ch:(_r + 1) * ch, :], in_=slab_k)
                else:
                    nc.gpsimd.collective_compute(
                        "AllGather", mybir.AluOpType.bypass,
                        replica_groups=[list(range(num_devices))],
                        ins=[slab_k], outs=[full_k],
                    )
                # stage chunk k: core r's sub-slab -> h_loc[r*NLOC + k*ch ...]
                nseg = max(COPY_SEGS // (agc * NCORES), 1)
                for _r in range(NCORES):
                    src = full_k[_r * ch:(_r + 1) * ch, :]
                    dst = h_loc[_r * NLOC + k * ch:_r * NLOC + (k + 1) * ch, :]
                    sseg = (ch + nseg - 1) // nseg
                    for si in range(nseg):
                        lo_r, hi_r = si * sseg, min((si + 1) * sseg, ch)
                        engs[ei % len(engs)].dma_start(out=dst[lo_r:hi_r, :],
                                                       in_=src[lo_r:hi_r, :])
                        ei += 1

            # --- layer 2 (h2 in bf16, ready for pooling matmuls) ---
            h2 = hbuf.tile([F, NLOC], BF, tag="h2")
            l2_tab = xbf_in if L2_FROM_X else h_loc
            _emit_cheb(nc, tc, ctx, pl, pools, tiles, l2_tab, w2_t, b2_t,
                       h2[:], BF)

            # --- pooling: pooled[g, f] = sum_n M[n, g] h2[n, f] ---
            ps_pool = psp.tile([G, F], FP, tag="pspool")
            for i in range(NT):
                lo = i * P
                wdt = min(P, NLOC - lo)
                ps_t = pst.tile([P, F], BF, tag="pstb")
                nc.tensor.matmul(out=ps_t[:wdt, :], lhsT=h2[:, lo:lo + wdt],
                                 rhs=ident[:], start=True, stop=True,
                                 is_transpose=True)
                h2nm_t = mp.tile([P, F], BF, tag="hnm")
                nc.vector.tensor_copy(out=h2nm_t[:wdt, :], in_=ps_t[:wdt, :])
                m_t = mp.tile([P, G], BF, tag="mt")
                nc.vector.tensor_tensor(
                    out=m_t[:], in0=bat_t[:, i:i + 1].broadcast_to([P, G]),
                    in1=iota_g[:], op=mybir.AluOpType.is_equal)
                nc.tensor.matmul(out=ps_pool[:], lhsT=m_t[:wdt, :],
                                 rhs=h2nm_t[:wdt, :],
                                 start=(i == 0), stop=(i == NT - 1))
            pooled_t = sml.tile([G, F], FP, tag="pooled")
            nc.scalar.copy(out=pooled_t[:], in_=ps_pool[:])
            prod_t = sml.tile([G, F], FP, tag="prod")
            nc.vector.tensor_mul(prod_t[:], pooled_t[:], lw_t[:G, :])
            outp_t = sml.tile([G, 1], FP, tag="outp")
            nc.vector.tensor_reduce(out=outp_t[:], in_=prod_t[:],
                                    axis=mybir.AxisListType.X,
                                    op=mybir.AluOpType.add)
            nc.sync.dma_start(out=out_d[:], in_=outp_t[:])
        # reset Q7 ucode for the next NEFF execution; _finalize_bir moves this
        # after the last DMAGatherAnt (Tile schedules dep-free insts eagerly)
        nc.gpsimd.load_library(library_config.standard)

    _fix_reload_order(nc)
    return nc


def mult_op():
    return mybir.AluOpType.mult


def add_op():
    return mybir.AluOpType.add


def max_op():
    return mybir.AluOpType.max


# ---------------------------------------------------------------------------
# Entry point
# ---------------------------------------------------------------------------
def _prepare(inputs, G=100):
    x = np.asarray(inputs["x"], dtype=np.float32)
    edge_index = np.asarray(inputs["edge_index"])
    batch = np.asarray(inputs["batch"])
    W1 = np.asarray(inputs["W1"], dtype=np.float32)
    b1 = np.asarray(inputs["b1"], dtype=np.float32)
    W2 = np.asarray(inputs["W2"], dtype=np.float32)
    b2 = np.asarray(inputs["b2"], dtype=np.float32)
    gamma = np.asarray(inputs["gamma"], dtype=np.float32)
    beta = np.asarray(inputs["beta"], dtype=np.float32)
    linW = np.asarray(inputs["linW"], dtype=np.float32)

    pl = _plan(edge_index, batch, x, G)
    NLOC = pl.NLOC
    NT = (NLOC + P - 1) // P
    w1s = np.concatenate([W1[0], W1[1]], axis=0).astype(NPBF)  # [128, 64]
    w2s = np.concatenate([W2[0], W2[1]], axis=0).astype(NPBF)
    lw_rep = np.tile(linW[:, 0][None, :], (P, 1)).astype(np.float32)
    xbf = np.zeros((x.shape[0], TW), NPBF)
    xbf[:, :F] = x.astype(NPBF)
    in_maps = []
    for m in range(NCORES):
        sl = slice(m * NLOC, (m + 1) * NLOC)
        bat_loc = np.full((P, NT), -1.0, np.float32)
        bl = pl.batch[sl].astype(np.float32)
        for i in range(NT):
            seg = bl[i * P:(i + 1) * P]
            bat_loc[:len(seg), i] = seg
        in_maps.append({
            "xbf_in": xbf,
            "xT_in": np.ascontiguousarray(x[sl].T).astype(NPBF),
            "idx_in": pl.idx_full[m],
            "w_in": pl.w_pc[m].astype(NPBF),
            "rel_in": pl.rel_pc[m].astype(NPBF),
            "w1_in": w1s, "w2_in": w2s,
            "b1_in": b1[:, None], "b2_in": b2[:, None],
            "gam_in": gamma[:, None], "bet_in": beta[:, None],
            "lw_in": lw_rep,
            "bat_in": bat_loc.astype(NPBF),
        })
    return pl, in_maps


def run_gnn(inputs, trace=False):
    linb = np.asarray(inputs["linb"], dtype=np.float32)
    pl, in_maps = _prepare(inputs)
    nc = _build_program(pl)
    _finalize_bir(nc)
    res = run_bass_kernel_spmd(nc, in_maps, list(range(NCORES)), trace=trace)
    partial = sum(res.results[m]["out_d"] for m in range(NCORES))
    out = partial / np.maximum(pl.cnts, 1.0)[:, None] + linb[None, :]
    return out.astype(np.float32), res


def kernel(**inputs):
    out, _ = run_gnn(inputs, trace=False)
    return out


# revision 45
# speedup vs baseline: 1.4499x; 1.4499x over previous
"""ChebConv(K=2) x2 + BatchNorm + LeakyReLU + global_mean_pool + linear head
on 8 Trainium2 NeuronCores.

Sharding: edges partitioned by destination node (col) across cores; node
features replicated in DRAM (bf16) for per-edge gathering; per-graph pooling
via one-hot matmul built on-device, partials combined on host.

bf16 data path: gather tables, scatter one-hot/weights, and all matmuls run
in bf16 (PSUM accumulation fp32); BatchNorm stats and the final head stay
fp32.

Self-contained: only needs the container's `concourse` package.
"""
import math
import numpy as np
from contextlib import ExitStack

import ml_dtypes

import concourse.bass as bass
import concourse.tile as tile
from concourse import mybir, library_config
from concourse.bass_utils import run_bass_kernel_spmd
from concourse.masks import make_identity

P = 128          # partitions / edges per chunk
BQ = 8           # max chunks per gather call (pow2 blocks)
SUB = 32         # dst subtile width (sel width)
SPW = 15         # subtiles per PSUM window (480 dst nodes + 32 slack = 512)
IDX_SPLIT = 32768  # int16 gather index limit -> two source groups
F = 64           # feature width (both layers)
NCORES = 8
NOGATHER = False  # differential profiling switch
FREERUN = False   # diagnostic: issue gathers but feed matmuls from a dummy
PREFETCH = 0      # extra windows of gather blocks issued ahead
L2_FROM_X = False  # diagnostic: layer-2 gathers read the x table, not h_full
COPY_SEGS = 8      # h_full -> h_loc staging copy segments (round-robin engines)
AG_CHUNKS = 1      # AllGather split count (pipelines collective with staging)
NQUEUES = 4      # SWDGE queues (max 4)
VAL_BUFS = 24    # gather tile pool depth
DMA_SCRATCH = 16384  # SWDGE descriptor ring bytes (//16 = ring descs)
SINGLE_PACKET = True

FP = mybir.dt.float32
BF = mybir.dt.bfloat16
I16 = mybir.dt.int16
NPBF = ml_dtypes.bfloat16
TW = 128         # padded gather-table row width (stride 256B in bf16)


def _dma_gather_raw(eng, out_ap, in_ap, idxs_ap, num_idxs, num_idxs_reg,
                    elem_size, elem_step, queue_num=0):
    """dma_gather with elem_size_bytes=128 (sub-256B payload) on a 256B-stride
    table. Mirrors BassGpSimd.dma_gather minus its elem_size%256 assert (the
    stride, which the ISA encodes in 256B units, stays a 256B multiple)."""
    from concourse import ap_utils
    from concourse.bass import MemorySpace

    assert idxs_ap.dtype == mybir.dt.int16
    assert in_ap.dtype == out_ap.dtype
    assert in_ap.space == MemorySpace.DRAM
    assert idxs_ap.space == MemorySpace.SBUF
    assert out_ap.space == MemorySpace.SBUF
    assert ap_utils.ap_is_contiguous(out_ap.ap[1:])
    assert ap_utils.ap_is_contiguous(idxs_ap.ap[1:])
    assert in_ap.ap[-1][1] == out_ap.ap[-1][1] == elem_size
    assert in_ap.ap[0][0] == elem_step
    stride_bytes = elem_step * mybir.dt.size(in_ap.dtype)
    assert stride_bytes % 256 == 0
    stride_bytes_256 = stride_bytes // 256
    _in_ap = eng.lower_ap_dma(in_ap, for_custom_bir_dma=True)
    inst = eng.add_instruction(
        mybir.InstDMAGatherAnt(
            name=eng.bass.get_next_instruction_name(),
            ins=[*_in_ap, eng.lower_ap(idxs_ap),
                 eng.lower_val_access(eng.to_reg(num_idxs_reg))],
            outs=[eng.lower_ap(out_ap)],
            transpose=False,
            num_idxs=num_idxs,
            elem_size=elem_size,
            stride_bytes_256=stride_bytes_256,
            gen_mode=0,
            single_packet=SINGLE_PACKET,
            queue_num=queue_num,
            sbuf_tokens_per_rank=0,
            sbuf_free_dim_per_rank=0,
            sbuf_free_dim_pad_per_rank=0,
            sbuf_byte_offset=0,
        )
    )
    return inst


# ---------------------------------------------------------------------------
# BIR post-passes (this container's walrus accepts only one sync wait per
# instruction, and never lowers InstPseudoReloadLibraryIndex itself).
# ---------------------------------------------------------------------------
_CTR = [0]


def _fix_reload_order(nc):
    """Tile schedules dep-free instructions eagerly; move the final
    standard-lib reload after the last DMAGatherAnt."""
    for f in nc.m.functions:
        for bb in f.blocks:
            insts = list(bb.instructions)
            std_i = [i for i, it in enumerate(insts)
                     if getattr(it, "op_name", None) == "PseudoReloadLibraryIndex"
                     and it.lib_index == 0]
            gat_i = [i for i, it in enumerate(insts)
                     if type(it).__name__ == "InstDMAGatherAnt"]
            if std_i and gat_i and std_i[0] < gat_i[-1]:
                reload_inst = insts.pop(std_i[0])
                insts.insert(gat_i[-1], reload_inst)
                bb.instructions = insts


def _finalize_bir(nc):
    for f in nc.m.functions:
        for bb in f.blocks:
            out = []
            changed = False
            for inst in bb.instructions:
                if getattr(inst, "op_name", None) == "PseudoReloadLibraryIndex":
                    instr = [0] * 64
                    instr[0] = 223
                    instr[1] = 16
                    instr[12] = 2
                    instr[16] = inst.lib_index
                    inst.instr = instr
                si = inst.sync_info
                if si is not None and si.on_wait is not None and len(si.on_wait) > 1:
                    changed = True
                    for w in si.on_wait[:-1]:
                        _CTR[0] += 1
                        nop = mybir.InstNoOp(
                            name=f"waitnop-{_CTR[0]}",
                            engine=inst.engine,
                            sync_info=mybir.SyncInfo(on_wait=[w], on_update=[]),
                        )
                        out.append(nop)
                    inst.sync_info = mybir.SyncInfo(
                        on_wait=[si.on_wait[-1]], on_update=si.on_update
                    )
                out.append(inst)
            if changed:
                bb.instructions = out


# ---------------------------------------------------------------------------
# Host-side planning: bucket edges by (core, dst subtile, src group), build
# the static chunk layout (max chunk count across cores per bucket) and the
# per-core packed idx/w/rel arrays.
# ---------------------------------------------------------------------------
class Plan:
    pass


def _plan(edge_index, batch, x, G):
    N = x.shape[0]
    E = edge_index.shape[1]
    NLOC = (N + NCORES - 1) // NCORES
    assert N == NLOC * NCORES, "node count must split evenly"
    NSUB = (NLOC + SUB - 1) // SUB
    NWIN = (NSUB + SPW - 1) // SPW

    row = np.asarray(edge_index[0], dtype=np.int64)
    col = np.asarray(edge_index[1], dtype=np.int64)
    deg = np.bincount(row, minlength=N).astype(np.float64)
    dis = np.where(deg > 0, deg ** -0.5, 0.0)
    w_all = (-(dis[row] * dis[col])).astype(np.float32)

    core_of = col // NLOC
    local = col - core_of * NLOC
    sub = local // SUB
    grp = (row >= IDX_SPLIT).astype(np.int64)

    key = (core_of * NSUB + sub) * 2 + grp
    counts = np.bincount(key, minlength=NCORES * NSUB * 2).reshape(NCORES, NSUB, 2)
    K = np.ceil(counts.max(axis=0) / P).astype(np.int64)  # [NSUB, 2]

    # chunk column layout ordered (group, window, subtile): each group's
    # columns are contiguous so gather blocks pack to the full BQ*P
    # descriptor budget regardless of window boundaries
    col_index = np.zeros((NSUB, 2), np.int64)
    spans = []  # (wi, g, col_start, ncols)
    group_base = {}
    T = 0
    for g in (0, 1):
        group_base[g] = T
        for wi in range(NWIN):
            subs = range(wi * SPW, min((wi + 1) * SPW, NSUB))
            start = T
            for s in subs:
                col_index[s, g] = T
                T += K[s, g]
            spans.append((wi, g, start, T - start))
    group_size = {0: group_base[1], 1: T - group_base[1]}

    # per-edge placement
    order = np.argsort(key, kind="stable")
    kk = key[order]
    bucket_first = np.r_[0, np.flatnonzero(np.diff(kk)) + 1]
    sizes = np.diff(np.r_[bucket_first, E])
    j = np.arange(E) - np.repeat(bucket_first, sizes)  # rank within bucket

    m_o = core_of[order]
    c_o = col_index[sub[order], grp[order]] + j // P
    p_o = j % P
    idx_loc = np.where(grp == 0, row, row - IDX_SPLIT).astype(np.int16)[order]
    w_o = w_all[order]
    rel_o = (local - sub * SUB).astype(np.float32)[order]

    w_pc = np.zeros((NCORES, P, T), np.float32)
    rel_pc = np.zeros((NCORES, P, T), np.float32)
    idx_pc = np.zeros((NCORES, 16, 8 * T), np.int16)
    w_pc[m_o, p_o, c_o] = w_o
    rel_pc[m_o, p_o, c_o] = rel_o
    idx_pc[m_o, p_o % 16, 8 * c_o + p_o // 16] = idx_loc
    idx_full = np.tile(idx_pc, (1, 8, 1))  # [NCORES, 128, 8T]

    pl = Plan()
    pl.N, pl.E, pl.G = N, E, G
    pl.NLOC, pl.NSUB, pl.NWIN, pl.T = NLOC, NSUB, NWIN, T
    pl.K, pl.col_index, pl.spans = K, col_index, spans
    pl.group_base, pl.group_size = group_base, group_size
    pl.w_pc, pl.rel_pc, pl.idx_full = w_pc, rel_pc, idx_full
    pl.empty_subs = [s for s in range(NSUB) if K[s, 0] + K[s, 1] == 0]
    pl.batch = np.asarray(batch, dtype=np.int64)
    pl.cnts = np.bincount(pl.batch, minlength=G).astype(np.float32)
    return pl


# ---------------------------------------------------------------------------
# Device program
# ---------------------------------------------------------------------------
def _emit_cheb(nc, tc, ctx, pl, pools, tiles, table_ap, wstack_t, bias_t, h_out,
               h_dtype):
    """One Cheb layer: scatter (gather + sel matmuls into PSUM windows) into
    stacked[64:128], then dense matmul with [x_T; Tx_T] and bias add into
    h_out[64, NLOC]."""
    NLOC, NSUB, NWIN, K = pl.NLOC, pl.NSUB, pl.NWIN, pl.K
    valp, selp, psw, psd = pools["val"], pools["sel"], pools["psw"], pools["psd"]
    stacked, idx_t, w_t, rel_t, iota_f = (
        tiles["stacked"], tiles["idx"], tiles["w"], tiles["rel"], tiles["iota_f"],
    )

    spans_by_win = {}
    for (wi, g, start, ncols) in pl.spans:
        spans_by_win.setdefault(wi, []).append((g, start, ncols))
    gbase, gsize = pl.group_base, pl.group_size

    # gather blocks stream over each group's contiguous column range in
    # full-BQ chunks, decoupled from window boundaries; issued lazily when
    # the first window touching them is processed
    blocks = {}  # (g, bid) -> (abs base col, tile)

    def ensure_block(g, bid):
        if (g, bid) in blocks:
            return
        b0 = gbase[g] + bid * BQ
        bw = min(BQ, gbase[g] + gsize[g] - b0)
        bt = valp.tile([P, BQ, F], BF, tag="val")
        if NOGATHER:
            nc.vector.memset(bt[:, :bw, :], 0.5)
        else:
            src = table_ap[:, :F] if g == 0 else table_ap[IDX_SPLIT:, :F]
            _dma_gather_raw(
                nc.gpsimd, out_ap=bt[:, :bw, :], in_ap=src,
                idxs_ap=idx_t[:, 8 * b0: 8 * (b0 + bw)],
                num_idxs=bw * P, num_idxs_reg=tiles["ni_regs"][bw * P],
                elem_size=F, elem_step=TW,
                queue_num=tiles["qrr"][0] % NQUEUES,
            )
        tiles["qrr"][0] += 1
        blocks[(g, bid)] = (b0, bt)

    dummy = None
    if FREERUN:
        dummy = valp.tile([P, BQ, F], BF, tag="dummy")
        nc.vector.memset(dummy[:], 0.5)

    for wi in range(NWIN):
        ps_w = psw.tile([F, 512], FP, tag="psw")
        win_subs = range(wi * SPW, min((wi + 1) * SPW, NSUB))
        sel_tiles = {}
        for wj in range(wi, min(wi + 1 + PREFETCH, NWIN)):
            for (g, start, ncols) in spans_by_win[wj]:
                if ncols == 0:
                    continue
                r0 = start - gbase[g]
                for bid in range(r0 // BQ, -(-(r0 + ncols) // BQ)):
                    ensure_block(g, bid)
        for (g, start, ncols) in spans_by_win[wi]:
            if ncols == 0:
                continue
            sel_t = selp.tile([P, ncols, SUB], BF, tag="sel")
            nc.vector.tensor_tensor(
                out=sel_t[:],
                in0=rel_t[:, start:start + ncols, None].broadcast_to([P, ncols, SUB]),
                in1=iota_f[:, None, :].broadcast_to([P, ncols, SUB]),
                op=mybir.AluOpType.is_equal,
            )
            nc.vector.tensor_tensor(
                out=sel_t[:], in0=sel_t[:],
                in1=w_t[:, start:start + ncols, None].broadcast_to([P, ncols, SUB]),
                op=mybir.AluOpType.mult,
            )
            sel_tiles[g] = (start, sel_t)

        # chunk matmuls, accumulate into the window PSUM; the first matmul
        # per subtile carries start=True (zero-initializes its PSUM slice)
        mms = []
        for s in win_subs:
            boff = SUB * (s - wi * SPW)
            for g in (0, 1):
                if K[s, g] == 0 or g not in sel_tiles:
                    continue
                for r in range(K[s, g]):
                    mms.append((s, boff, g, pl.col_index[s, g] + r))
        started = set()
        for i, (s, boff, g, cc) in enumerate(mms):
            b0, bt = blocks[(g, (cc - gbase[g]) // BQ)]
            if FREERUN:
                lhs = dummy[:, (cc - b0) % BQ, :]
            else:
                lhs = bt[:, cc - b0, :]
            span_start, sel_t = sel_tiles[g]
            nc.tensor.matmul(
                out=ps_w[:, boff:boff + SUB],
                lhsT=lhs, rhs=sel_t[:, cc - span_start, :],
                start=(s not in started), stop=(i == len(mms) - 1),
            )
            started.add(s)
        # copy the window's real columns into stacked[64:128]
        lo = wi * SPW * SUB
        hi = min(lo + SPW * SUB, NLOC)
        nc.scalar.copy(out=stacked[F:2 * F, lo:hi], in_=ps_w[:, :hi - lo])
        for s in pl.empty_subs:
            if wi * SPW <= s < min((wi + 1) * SPW, NSUB):
                slo, shi = s * SUB, min((s + 1) * SUB, NLOC)
                nc.vector.memset(stacked[F:2 * F, slo:shi], 0.0)

    # dense: h = Wstack.T @ [x_T; Tx_T] + b
    nspan = 512
    for i in range(0, NLOC, nspan):
        wdt = min(nspan, NLOC - i)
        ps_d = psd.tile([F, 512], FP, tag="psd")
        nc.tensor.matmul(out=ps_d[:, :wdt], lhsT=wstack_t[:],
                         rhs=stacked[:, i:i + wdt], start=True, stop=True)
        nc.scalar.activation(h_out[:, i:i + wdt], ps_d[:, :wdt],
                             mybir.ActivationFunctionType.Identity, bias=bias_t[:])


def _build_program(pl, num_devices=NCORES, no_cc=False, repeat=1):
    N, NLOC, G, T = pl.N, pl.NLOC, pl.G, pl.T
    NT = (NLOC + P - 1) // P
    nc = bass.Bass("TRN2", target_bir_lowering=False, debug=False,
                   num_devices=num_devices, num_swdge_queues=NQUEUES,
                   dynamic_dma_scratch_size=DMA_SCRATCH)

    xbf_in = nc.dram_tensor("xbf_in", [N, TW], BF, kind="ExternalInput").ap()
    xT_in = nc.dram_tensor("xT_in", [F, NLOC], BF, kind="ExternalInput").ap()
    idx_in = nc.dram_tensor("idx_in", [P, 8 * T], I16, kind="ExternalInput").ap()
    w_in = nc.dram_tensor("w_in", [P, T], BF, kind="ExternalInput").ap()
    rel_in = nc.dram_tensor("rel_in", [P, T], BF, kind="ExternalInput").ap()
    w1_in = nc.dram_tensor("w1_in", [2 * F, F], BF, kind="ExternalInput").ap()
    w2_in = nc.dram_tensor("w2_in", [2 * F, F], BF, kind="ExternalInput").ap()
    b1_in = nc.dram_tensor("b1_in", [F, 1], FP, kind="ExternalInput").ap()
    b2_in = nc.dram_tensor("b2_in", [F, 1], FP, kind="ExternalInput").ap()
    gam_in = nc.dram_tensor("gam_in", [F, 1], FP, kind="ExternalInput").ap()
    bet_in = nc.dram_tensor("bet_in", [F, 1], FP, kind="ExternalInput").ap()
    lw_in = nc.dram_tensor("lw_in", [P, F], FP, kind="ExternalInput").ap()
    bat_in = nc.dram_tensor("bat_in", [P, NT], BF, kind="ExternalInput").ap()
    out_d = nc.dram_tensor("out_d", [G, 1], FP, kind="ExternalOutput").ap()

    h_slab = nc.dram_tensor("h_slab", [NLOC, TW], BF).ap()
    h_full = nc.dram_tensor("h_full", [N, TW], BF, addr_space="Shared").ap()
    h_loc = nc.dram_tensor("h_loc", [N, TW], BF).ap()
    st_in = nc.dram_tensor("st_in", [F, 2], FP).ap()
    st_out = nc.dram_tensor("st_out", [F, 2], FP, addr_space="Shared").ap()

    with tile.TileContext(nc) as tc, ExitStack() as ctx:
        cst = ctx.enter_context(tc.tile_pool(name="cst", bufs=1))
        big = ctx.enter_context(tc.tile_pool(name="big", bufs=1))
        hbuf = ctx.enter_context(tc.tile_pool(name="hbuf", bufs=1))
        valp = ctx.enter_context(tc.tile_pool(name="valp", bufs=VAL_BUFS))
        selp = ctx.enter_context(tc.tile_pool(name="selp", bufs=4))
        mp = ctx.enter_context(tc.tile_pool(name="mp", bufs=2))
        sml = ctx.enter_context(tc.tile_pool(name="sml", bufs=1))
        psw = ctx.enter_context(tc.tile_pool(name="psw", bufs=4, space="PSUM"))
        psd = ctx.enter_context(tc.tile_pool(name="psd", bufs=1, space="PSUM"))
        pst = ctx.enter_context(tc.tile_pool(name="pst", bufs=2, space="PSUM"))
        psp = ctx.enter_context(tc.tile_pool(name="psp", bufs=1, space="PSUM"))
        pools = {"val": valp, "sel": selp, "psw": psw, "psd": psd}

        # --- constants & inputs (standard ucode ops before mlp lib load) ---
        iota_i = cst.tile([P, SUB], mybir.dt.int32)
        nc.gpsimd.iota(iota_i[:], pattern=[[1, SUB]], base=0, channel_multiplier=0)
        iota_f = cst.tile([P, SUB], BF)
        nc.vector.tensor_copy(out=iota_f[:], in_=iota_i[:])
        iota_gi = cst.tile([P, G], mybir.dt.int32)
        nc.gpsimd.iota(iota_gi[:], pattern=[[1, G]], base=0, channel_multiplier=0)
        iota_g = cst.tile([P, G], BF)
        nc.vector.tensor_copy(out=iota_g[:], in_=iota_gi[:])
        ident = cst.tile([F, F], BF)
        make_identity(nc, ident[:])
        nc.gpsimd.load_library(library_config.mlp)

        idx_t = cst.tile([P, 8 * T], I16)
        nc.sync.dma_start(out=idx_t[:], in_=idx_in[:])
        w_t = cst.tile([P, T], BF)
        nc.sync.dma_start(out=w_t[:], in_=w_in[:])
        rel_t = cst.tile([P, T], BF)
        nc.sync.dma_start(out=rel_t[:], in_=rel_in[:])
        w1_t = cst.tile([2 * F, F], BF)
        nc.sync.dma_start(out=w1_t[:], in_=w1_in[:])
        w2_t = cst.tile([2 * F, F], BF)
        nc.sync.dma_start(out=w2_t[:], in_=w2_in[:])
        b1_t = cst.tile([F, 1], FP)
        nc.sync.dma_start(out=b1_t[:], in_=b1_in[:])
        b2_t = cst.tile([F, 1], FP)
        nc.sync.dma_start(out=b2_t[:], in_=b2_in[:])
        gam_t = cst.tile([F, 1], FP)
        nc.sync.dma_start(out=gam_t[:], in_=gam_in[:])
        bet_t = cst.tile([F, 1], FP)
        nc.sync.dma_start(out=bet_t[:], in_=bet_in[:])
        lw_t = cst.tile([P, F], FP)
        nc.sync.dma_start(out=lw_t[:], in_=lw_in[:])
        bat_t = cst.tile([P, NT], BF)
        nc.sync.dma_start(out=bat_t[:], in_=bat_in[:])

        stacked = big.tile([P, NLOC], BF)
        ni_regs = {}
        for r in range(1, BQ + 1):
            v = r * P
            reg = nc.gpsimd.alloc_register(f"ni{v}")
            nc.gpsimd.reg_mov(reg, v)
            ni_regs[v] = reg
        tiles = {"stacked": stacked, "idx": idx_t, "w": w_t, "rel": rel_t,
                 "iota_f": iota_f, "ni_regs": ni_regs, "qrr": [0]}

        for _rep in range(repeat):
            # --- layer 1 ---
            nc.sync.dma_start(out=stacked[:F, :], in_=xT_in[:])
            h_pre = hbuf.tile([F, NLOC], FP, tag="hpre")
            _emit_cheb(nc, tc, ctx, pl, pools, tiles, xbf_in, w1_t, b1_t,
                       h_pre[:], FP)

            # --- BN stats + AllReduce ---
            sum_t = sml.tile([F, 1], FP, tag="sum")
            nc.vector.tensor_reduce(out=sum_t[:], in_=h_pre[:],
                                    axis=mybir.AxisListType.X,
                                    op=mybir.AluOpType.add)
            scratch = hbuf.tile([F, NLOC], FP, tag="scratch2")
            sumsq_t = sml.tile([F, 1], FP, tag="sumsq")
            nc.scalar.activation(scratch[:], h_pre[:],
                                 mybir.ActivationFunctionType.Square,
                                 accum_out=sumsq_t[:])
            st_t = sml.tile([F, 2], FP, tag="st")
            nc.vector.tensor_copy(out=st_t[:, 0:1], in_=sum_t[:])
            nc.vector.tensor_copy(out=st_t[:, 1:2], in_=sumsq_t[:])
            nc.sync.dma_start(out=st_in[:], in_=st_t[:])
            if no_cc:
                nc.sync.dma_start(out=st_out[:], in_=st_in[:])
            else:
                nc.gpsimd.collective_compute(
                    "AllReduce", mybir.AluOpType.add,
                    replica_groups=[list(range(num_devices))],
                    ins=[st_in[:]], outs=[st_out[:]],
                )
            str_t = sml.tile([F, 2], FP, tag="str")
            nc.sync.dma_start(out=str_t[:], in_=st_out[:])

            # s = gamma * rsqrt(var + eps); t = beta - mu * s
            invN = 1.0 / float(N)
            mu_t = sml.tile([F, 1], FP, tag="mu")
            nc.vector.tensor_scalar_mul(mu_t[:], str_t[:, 0:1], invN)
            msq_t = sml.tile([F, 1], FP, tag="msq")
            nc.vector.tensor_scalar_mul(msq_t[:], str_t[:, 1:2], invN)
            var_t = sml.tile([F, 1], FP, tag="var")
            nc.vector.scalar_tensor_tensor(out=var_t[:], in0=mu_t[:],
                                           scalar=-1.0, in1=mu_t[:],
                                           op0=mult_op(), op1=mult_op())
            nc.vector.tensor_add(var_t[:], var_t[:], msq_t[:])
            eps_t = sml.tile([F, 1], FP, tag="eps")
            nc.vector.memset(eps_t[:], 1e-5)
            sd_t = sml.tile([F, 1], FP, tag="sd")
            nc.scalar.activation(sd_t[:], var_t[:],
                                 mybir.ActivationFunctionType.Sqrt,
                                 bias=eps_t[:])
            rs_t = sml.tile([F, 1], FP, tag="rs")
            nc.vector.reciprocal(rs_t[:], sd_t[:])
            s_t = sml.tile([F, 1], FP, tag="s")
            nc.vector.tensor_mul(s_t[:], gam_t[:], rs_t[:])
            t_t = sml.tile([F, 1], FP, tag="t")
            nc.vector.tensor_mul(t_t[:], mu_t[:], s_t[:])
            nc.vector.tensor_sub(t_t[:], bet_t[:], t_t[:])

            # h = lrelu(h_pre * s + t) -> stacked[:64] (bf16)
            z_t = hbuf.tile([F, NLOC], FP, tag="scratch2")
            nc.vector.tensor_scalar(out=z_t[:], in0=h_pre[:], scalar1=s_t[:],
                                    scalar2=t_t[:], op0=mult_op(),
                                    op1=add_op())
            nc.vector.scalar_tensor_tensor(out=stacked[:F, :], in0=z_t[:],
                                           scalar=0.01, in1=z_t[:],
                                           op0=mult_op(), op1=max_op())

            # transpose h -> h_slab (node-major bf16) and AllGather; the
            # bf16 transpose writes PSUM in bf16, so DMA straight from PSUM
            for i in range(0, NLOC, P):
                wdt = min(P, NLOC - i)
                ps_b = pst.tile([P, F], BF, tag="pstb")
                nc.tensor.matmul(out=ps_b[:wdt, :], lhsT=stacked[:F, i:i + wdt],
                                 rhs=ident[:], start=True, stop=True,
                                 is_transpose=True)
                hnm_t = mp.tile([P, F], BF, tag="hnm")
                nc.vector.tensor_copy(out=hnm_t[:wdt, :], in_=ps_b[:wdt, :])
                nc.sync.dma_start(out=h_slab[i:i + wdt, :F], in_=hnm_t[:wdt, :])
            # AllGather h, then stage the table into local DRAM (Shared-space
            # random reads are ~5x slower than local DRAM). AG_CHUNKS > 1
            # pipelines the collective chunks with the staging copies.
            agc = AG_CHUNKS
            ch = NLOC // agc
            assert NLOC % agc == 0
            engs = [nc.sync, nc.scalar]
            ei = 0
            for k in range(agc):
                slab_k = h_slab[k * ch:(k + 1) * ch, :]
                full_k = h_full[k * ch * NCORES:(k + 1) * ch * NCORES, :]
                if no_cc:
                    for _r in range(NCORES):
                        nc.sync.dma_start(
                            out=full_k[_r * ch:(_r + 1) * ch, :], in_=slab_k)
                else:
                    nc.gpsimd.collective_compute(
                        "AllGather", mybir.AluOpType.bypass,
                        replica_groups=[list(range(num_devices))],
                        ins=[slab_k], outs=[full_k],
                    )
                # stage chunk k: core r's sub-slab -> h_loc[r*NLOC + k*ch ...]
                nseg = max(COPY_SEGS // (agc * NCORES), 1)
                for _r in range(NCORES):
                    src = full_k[_r * ch:(_r + 1) * ch, :]
                    dst = h_loc[_r * NLOC + k * ch:_r * NLOC + (k + 1) * ch, :]
                    sseg = (ch + nseg - 1) // nseg
                    for si in range(nseg):
                        lo_r, hi_r = si * sseg, min((si + 1) * sseg, ch)
                        engs[ei % len(engs)].dma_start(out=dst[lo_r:hi_r, :],
                                                       in_=src[lo_r:hi_r, :])
                        ei += 1

            # --- layer 2 (h2 in bf16, ready for pooling matmuls) ---
            h2 = hbuf.tile([F, NLOC], BF, tag="h2")
            l2_tab = xbf_in if L2_FROM_X else h_loc
            _emit_cheb(nc, tc, ctx, pl, pools, tiles, l2_tab, w2_t, b2_t,
                       h2[:], BF)

            # --- pooling: pooled[g, f] = sum_n M[n, g] h2[n, f] ---
            ps_pool = psp.tile([G, F], FP, tag="pspool")
            for i in range(NT):
                lo = i * P
                wdt = min(P, NLOC - lo)
                ps_t = pst.tile([P, F], BF, tag="pstb")
                nc.tensor.matmul(out=ps_t[:wdt, :], lhsT=h2[:, lo:lo + wdt],
                                 rhs=ident[:], start=True, stop=True,
                                 is_transpose=True)
                h2nm_t = mp.tile([P, F], BF, tag="hnm")
                nc.vector.tensor_copy(out=h2nm_t[:wdt, :], in_=ps_t[:wdt, :])
                m_t = mp.tile([P, G], BF, tag="mt")
                nc.vector.tensor_tensor(
                    out=m_t[:], in0=bat_t[:, i:i + 1].broadcast_to([P, G]),
                    in1=iota_g[:], op=mybir.AluOpType.is_equal)
                nc.tensor.matmul(out=ps_pool[:], lhsT=m_t[:wdt, :],
                                 rhs=h2nm_t[:wdt, :],
                                 start=(i == 0), stop=(i == NT - 1))
            pooled_t = sml.tile([G, F], FP, tag="pooled")
            nc.scalar.copy(out=pooled_t[:], in_=ps_pool[:])
            prod_t = sml.tile([G, F], FP, tag="prod")
            nc.vector.tensor_mul(prod_t[:], pooled_t[:], lw_t[:G, :])
            outp_t = sml.tile([G, 1], FP, tag="outp")
            nc.vector.tensor_reduce(out=outp_t[:], in_=prod_t[:],
                                    axis=mybir.AxisListType.X,
                                    op=mybir.AluOpType.add)
            nc.sync.dma_start(out=out_d[:], in_=outp_t[:])
        # reset Q7 ucode for the next NEFF execution; _finalize_bir moves this
        # after the last DMAGatherAnt (Tile schedules dep-free insts eagerly)
        nc.gpsimd.load_library(library_config.standard)

    _fix_reload_order(nc)
    return nc


def mult_op():
    return mybir.AluOpType.mult


def add_op():
    return mybir.AluOpType.add


def max_op():
    return mybir.AluOpType.max


# ---------------------------------------------------------------------------
# Entry point
# ---------------------------------------------------------------------------
def _prepare(inputs, G=100):
    x = np.asarray(inputs["x"], dtype=np.float32)
    edge_index = np.asarray(inputs["edge_index"])
    batch = np.asarray(inputs["batch"])
    W1 = np.asarray(inputs["W1"], dtype=np.float32)
    b1 = np.asarray(inputs["b1"], dtype=np.float32)
    W2 = np.asarray(inputs["W2"], dtype=np.float32)
    b2 = np.asarray(inputs["b2"], dtype=np.float32)
    gamma = np.asarray(inputs["gamma"], dtype=np.float32)
    beta = np.asarray(inputs["beta"], dtype=np.float32)
    linW = np.asarray(inputs["linW"], dtype=np.float32)

    pl = _plan(edge_index, batch, x, G)
    NLOC = pl.NLOC
    NT = (NLOC + P - 1) // P
    w1s = np.concatenate([W1[0], W1[1]], axis=0).astype(NPBF)  # [128, 64]
    w2s = np.concatenate([W2[0], W2[1]], axis=0).astype(NPBF)
    lw_rep = np.tile(linW[:, 0][None, :], (P, 1)).astype(np.float32)
    xbf = np.zeros((x.shape[0], TW), NPBF)
    xbf[:, :F] = x.astype(NPBF)
    in_maps = []
    for m in range(NCORES):
        sl = slice(m * NLOC, (m + 1) * NLOC)
        bat_loc = np.full((P, NT), -1.0, np.float32)
        bl = pl.batch[sl].astype(np.float32)
        for i in range(NT):
            seg = bl[i * P:(i + 1) * P]
            bat_loc[:len(seg), i] = seg
        in_maps.append({
            "xbf_in": xbf,
            "xT_in": np.ascontiguousarray(x[sl].T).astype(NPBF),
            "idx_in": pl.idx_full[m],
            "w_in": pl.w_pc[m].astype(NPBF),
            "rel_in": pl.rel_pc[m].astype(NPBF),
            "w1_in": w1s, "w2_in": w2s,
            "b1_in": b1[:, None], "b2_in": b2[:, None],
            "gam_in": gamma[:, None], "bet_in": beta[:, None],
            "lw_in": lw_rep,
            "bat_in": bat_loc.astype(NPBF),
        })
    return pl, in_maps


def run_gnn(inputs, trace=False):
    linb = np.asarray(inputs["linb"], dtype=np.float32)
    pl, in_maps = _prepare(inputs)
    nc = _build_program(pl)
    _finalize_bir(nc)
    res = run_bass_kernel_spmd(nc, in_maps, list(range(NCORES)), trace=trace)
    partial = sum(res.results[m]["out_d"] for m in range(NCORES))
    out = partial / np.maximum(pl.cnts, 1.0)[:, None] + linb[None, :]
    return out.astype(np.float32), res


def kernel(**inputs):
    out, _ = run_gnn(inputs, trace=False)
    return out
